# revision 10
# baseline (speedup 1.0000x reference)
"""Causal self-attention Trainium2 kernel (8 NeuronCores, tensor-parallel).

Sharding: core c handles batch b=c//2 and heads [8*(c%2) .. 8*(c%2)+8).
Each core computes QKV for its 8 heads, causal flash-style attention in
transposed (S^T) layout, and a partial output projection over its 512
head-features. Host sums the two half partials per batch and adds b_proj.

Datapath is bf16 (fp32 PSUM accumulation); attention is j-outer so the
output projection of query-block j overlaps the next block's attention.

Self-contained: only imports concourse / jax / numpy / ml_dtypes.
"""
import numpy as np
import ml_dtypes
import jax
from jax.sharding import Mesh, PartitionSpec
from jax.experimental.shard_map import shard_map

import concourse.bacc as bacc
import concourse.mybir as mybir
import concourse.tile as tile
from concourse.bass2jax import (_bass_exec_p, install_neuronx_cc_hook,
                                partition_id_tensor)

B, SEQ, D = 4, 2048, 1024
H, HD = 16, 64
NCORES = 8
P = 128
QB = 512            # q block
NQB = SEQ // QB     # 4
NKT = SEQ // P      # 16 k tiles
PAIRS = 4           # head pairs per core
FP32 = mybir.dt.float32
BF16 = mybir.dt.bfloat16
AF = mybir.ActivationFunctionType
ALU = mybir.AluOpType
BF = ml_dtypes.bfloat16


def build_nc(nreps: int = 1, debug: bool = False, rep_phase: str = "all"):
    nc = bacc.Bacc()
    xT_e = nc.dram_tensor("xT", [D, SEQ], BF16, kind="ExternalInput")
    wkq_e = nc.dram_tensor("wkq", [PAIRS, 8, P, 256], BF16, kind="ExternalInput")
    bkq_e = nc.dram_tensor("bkq", [P, 8], FP32, kind="ExternalInput")
    wv_e = nc.dram_tensor("wv", [8, P, 520], BF16, kind="ExternalInput")
    bv_e = nc.dram_tensor("bv", [1, 520], FP32, kind="ExternalInput")
    wproj_e = nc.dram_tensor("wproj", [PAIRS, P, D], BF16, kind="ExternalInput")
    mask_e = nc.dram_tensor("mask", [P, P], BF16, kind="ExternalInput")
    y_e = nc.dram_tensor("y", [SEQ, D], FP32, kind="ExternalOutput")
    if debug:
        sad = nc.dram_tensor("sad", [PAIRS, NQB, P, QB], FP32,
                             kind="ExternalOutput")

    with tile.TileContext(nc) as tc:
        with (
            tc.tile_pool(name="const", bufs=1) as constp,
            tc.tile_pool(name="kqv", bufs=1) as kqv,
            tc.tile_pool(name="ps", bufs=2, space="PSUM") as psp,      # scores: 2x2 banks
            tc.tile_pool(name="pv", bufs=2, space="PSUM") as pvp,      # pv accum: 2 banks
            tc.tile_pool(name="pj", bufs=2, space="PSUM") as pjp,      # kq/proj: 2 banks
        ):
            mask_t = constp.tile([P, P], BF16, name="mask_t")
            nc.sync.dma_start(out=mask_t[:], in_=mask_e[:])
            bkq_t = constp.tile([P, 8], FP32, name="bkq_t")
            nc.sync.dma_start(out=bkq_t[:], in_=bkq_e[:])
            bv_t = constp.tile([1, 520], FP32, name="bv_t")
            nc.sync.dma_start(out=bv_t[:], in_=bv_e[:])
            bv_bc = constp.tile([P, 520], FP32, name="bv_bc")
            nc.gpsimd.partition_broadcast(bv_bc[:], bv_t[:], channels=P)

            kT = [kqv.tile([P, SEQ], BF16, name=f"kT{p}") for p in range(PAIRS)]
            qT = [kqv.tile([P, SEQ], BF16, name=f"qT{p}") for p in range(PAIRS)]
            vt = [kqv.tile([P, 520], BF16, name=f"v{i}") for i in range(NKT)]

            def qkv_phase(_rep):
                # ================= Phase 1: QKV =================
                with (
                    tc.tile_pool(name=f"wts{_rep}", bufs=1) as wts,
                    tc.tile_pool(name=f"xt{_rep}", bufs=1) as xtp,
                ):
                    wkq_t = [[wts.tile([P, 256], BF16,
                                       name=f"wkq{_rep}_{p}_{c}")
                              for c in range(8)] for p in range(PAIRS)]
                    wv_t = [wts.tile([P, 520], BF16, name=f"wv{_rep}_{c}")
                            for c in range(8)]
                    # DMA priority: pair-0 weights, then x (full rows, 4KB
                    # lines), then the rest.
                    for c in range(8):
                        nc.sync.dma_start(out=wkq_t[0][c][:], in_=wkq_e[0, c])
                    xts = []
                    for c in range(8):
                        t = xtp.tile([P, SEQ], BF16, tag=f"xt{c}",
                                     name=f"xt{_rep}_{c}")
                        nc.sync.dma_start(out=t[:],
                                          in_=xT_e[c * P:(c + 1) * P, :])
                        xts.append(t)
                    for p in range(1, PAIRS):
                        for c in range(8):
                            nc.sync.dma_start(out=wkq_t[p][c][:],
                                              in_=wkq_e[p, c])
                    for c in range(8):
                        nc.sync.dma_start(out=wv_t[c][:], in_=wv_e[c])
                    for nb in range(NQB):
                        s0 = slice(nb * QB, (nb + 1) * QB)
                        for p in range(PAIRS):
                            for ec in range(2):
                                ps = pjp.tile([P, QB], FP32, tag="pj",
                                              name=f"kq{_rep}_{nb}_{p}_{ec}")
                                for c in range(8):
                                    nc.tensor.matmul(
                                        ps[:],
                                        wkq_t[p][c][:, ec * P:(ec + 1) * P],
                                        xts[c][:, s0], start=(c == 0),
                                        stop=(c == 7))
                                dst = (kT if ec == 0 else qT)[p]
                                nc.vector.tensor_scalar_add(
                                    dst[:, s0], ps[:],
                                    bkq_t[:, 2 * p + ec:2 * p + ec + 1])
                        for ntl in range(4):
                            nt = nb * 4 + ntl
                            tsl = slice(nb * QB + ntl * P,
                                        nb * QB + (ntl + 1) * P)
                            for half in range(2):
                                lo, hi = half * 260, (half + 1) * 260
                                psv = pvp.tile([P, QB], FP32, tag="pv",
                                               name=f"v{_rep}_{nt}_{half}ps")
                                for c in range(8):
                                    nc.tensor.matmul(
                                        psv[:, 0:260],
                                        xts[c][:, tsl],
                                        wv_t[c][:, lo:hi],
                                        start=(c == 0), stop=(c == 7))
                                nc.vector.tensor_tensor(
                                    vt[nt][:, lo:hi], psv[:, 0:260],
                                    bv_bc[:, lo:hi], ALU.add)

            def attn_phase(_rep):
                # ========= Phase 2+3: attention & projection (j-outer) =========
                with (
                    tc.tile_pool(name=f"sa{_rep}", bufs=2) as sap,
                    tc.tile_pool(name=f"ep{_rep}", bufs=1) as ep,
                    tc.tile_pool(name=f"rc{_rep}", bufs=4) as rcp,
                    tc.tile_pool(name=f"dv{_rep}", bufs=2) as dvp,
                    tc.tile_pool(name=f"w3{_rep}", bufs=1) as wts3,
                    tc.tile_pool(name=f"yp{_rep}", bufs=2) as yp,
                ):
                    wproj_t = [wts3.tile([P, D], BF16,
                                         name=f"wproj{_rep}_{p}")
                               for p in range(PAIRS)]
                    for p in range(PAIRS):
                        nc.sync.dma_start(out=wproj_t[p][:], in_=wproj_e[p])
                    for j in range(NQB):
                        saT = []
                        for p in range(PAIRS):
                            hA, hB = 2 * p, 2 * p + 1
                            sa_t = sap.tile([P, QB], BF16, tag=f"sa{p}",
                                            name=f"sa{_rep}_{p}_{j}")
                            saT.append(sa_t)
                            pvA = pvp.tile([65, QB], FP32, tag="pv",
                                           name=f"pvA{_rep}_{p}_{j}")
                            pvB = pvp.tile([65, QB], FP32, tag="pv",
                                           name=f"pvB{_rep}_{p}_{j}")
                            eAs, eBs = [], []
                            for g in range(2 * j + 2):
                                sA = psp.tile([P, 1024], FP32, tag="s",
                                              name=f"sA{_rep}_{p}_{j}_{g}")
                                sB = psp.tile([P, 1024], FP32, tag="s",
                                              name=f"sB{_rep}_{p}_{j}_{g}")
                                for t in range(2):
                                    i = 2 * g + t
                                    off = P * (i - 4 * j) if i >= 4 * j else 0
                                    ksl = slice(i * P, (i + 1) * P)
                                    qsl = slice(j * QB + off, (j + 1) * QB)
                                    osl = slice(t * QB + off, (t + 1) * QB)
                                    nc.tensor.matmul(
                                        sA[:, osl], kT[p][0:64, ksl],
                                        qT[p][0:64, qsl], start=True,
                                        stop=True, tile_position=(0, 0))
                                    nc.tensor.matmul(
                                        sB[:, osl], kT[p][64:P, ksl],
                                        qT[p][64:P, qsl], start=True,
                                        stop=True, tile_position=(64, 0))
                                eA = ep.tile([P, 1024], BF16, tag=f"e{g}",
                                             name=f"eA{_rep}_{p}_{j}_{g}")
                                eB = ep.tile([P, 1024], BF16, tag=f"f{g}",
                                             name=f"eB{_rep}_{p}_{j}_{g}")
                                eAs.append(eA)
                                eBs.append(eB)
                                nc.scalar.activation(eA[:], sA[:], AF.Exp,
                                                     scale=0.125)
                                nc.scalar.activation(eB[:], sB[:], AF.Exp,
                                                     scale=0.125)
                                for t in range(2):
                                    i = 2 * g + t
                                    if i >= 4 * j:
                                        c0 = t * QB + P * (i - 4 * j)
                                        for e_t in (eA, eB):
                                            nc.vector.tensor_mul(
                                                e_t[:, c0:c0 + P],
                                                e_t[:, c0:c0 + P], mask_t[:])
                            # consecutive accumulation chains per psum (no
                            # per-mm bank alternation)
                            for pv, es, h in ((pvA, eAs, hA), (pvB, eBs, hB)):
                                for g in range(2 * j + 2):
                                    for t in range(2):
                                        i = 2 * g + t
                                        off = (P * (i - 4 * j)
                                               if i >= 4 * j else 0)
                                        esl = slice(t * QB + off,
                                                    (t + 1) * QB)
                                        st, sp = (i == 0), (i == 4 * j + 3)
                                        nc.tensor.matmul(
                                            pv[:, off:QB],
                                            vt[i][:, 65 * h:65 * h + 65],
                                            es[g][:, esl], start=st, stop=sp)
                            recA = rcp.tile([1, QB], FP32, tag="rc",
                                            name=f"rA{_rep}_{p}_{j}")
                            recB = rcp.tile([1, QB], FP32, tag="rc",
                                            name=f"rB{_rep}_{p}_{j}")
                            nc.vector.reciprocal(recA[:], pvA[64:65, :])
                            nc.vector.reciprocal(recB[:], pvB[64:65, :])
                            divA = dvp.tile([64, QB], FP32, tag="divA",
                                            name=f"dvA{_rep}_{p}_{j}")
                            divB = dvp.tile([64, QB], FP32, tag="divB",
                                            name=f"dvB{_rep}_{p}_{j}")
                            nc.gpsimd.partition_broadcast(divA[:], recA[:],
                                                          channels=64)
                            nc.gpsimd.partition_broadcast(divB[:], recB[:],
                                                          channels=64)
                            nc.vector.tensor_tensor(
                                sa_t[0:64, :], pvA[0:64, :], divA[:],
                                ALU.mult)
                            nc.vector.tensor_tensor(
                                sa_t[64:P, :], pvB[0:64, :], divB[:],
                                ALU.mult)
                            if debug:
                                nc.sync.dma_start(
                                    out=sad[p, j],
                                    in_=sa_t[:].bitcast(mybir.dt.uint16))

                        # ---- projection for this q-block ----
                        for half in range(2):
                            for ntl in range(4):
                                psy = pjp.tile([P, QB], FP32, tag="pj",
                                               name=f"y{_rep}_{j}_{half}_{ntl}")
                                for p in range(PAIRS):
                                    nc.tensor.matmul(
                                        psy[:],
                                        saT[p][:, ntl * P:(ntl + 1) * P],
                                        wproj_t[p][:, half * QB:(half + 1) * QB],
                                        start=(p == 0), stop=(p == 3))
                                yt = yp.tile([P, QB], FP32, tag="y",
                                             name=f"yt{_rep}_{j}_{half}_{ntl}")
                                nc.vector.tensor_copy(yt[:], psy[:])
                                r0 = j * QB + ntl * P
                                nc.sync.dma_start(
                                    out=y_e[r0:r0 + P,
                                            half * QB:(half + 1) * QB],
                                    in_=yt[:])

            if rep_phase == "all":
                for r in range(nreps):
                    qkv_phase(r)
                    attn_phase(r)
            elif rep_phase == "attn":
                qkv_phase(0)
                for r in range(nreps):
                    attn_phase(r)
            else:
                for r in range(nreps):
                    qkv_phase(r)
                attn_phase(0)
    nc.finalize()
    return nc


def prep_core_inputs(core, x, W_kqv, b_kqv, W_proj):
    b, half = core // 2, core % 2
    heads = [8 * half + m for m in range(8)]
    xT = np.ascontiguousarray(np.asarray(x[b], np.float32).T).astype(BF)
    wkq = np.empty((PAIRS, 8, P, 256), np.float32)
    bkq = np.empty((P, 8), np.float32)
    for p in range(PAIRS):
        gA, gB = heads[2 * p], heads[2 * p + 1]
        blk = np.empty((D, 256), np.float32)
        blk[:, 0:64] = W_kqv[gA][:, 0:64]      # k_A
        blk[:, 64:128] = W_kqv[gB][:, 0:64]    # k_B
        blk[:, 128:192] = W_kqv[gA][:, 64:128]  # q_A
        blk[:, 192:256] = W_kqv[gB][:, 64:128]  # q_B
        wkq[p] = blk.reshape(8, P, 256)
        bkq[0:64, 2 * p] = b_kqv[gA][0:64]
        bkq[64:128, 2 * p] = b_kqv[gB][0:64]
        bkq[0:64, 2 * p + 1] = b_kqv[gA][64:128]
        bkq[64:128, 2 * p + 1] = b_kqv[gB][64:128]
    wv = np.zeros((D, 520), np.float32)
    bv = np.zeros((1, 520), np.float32)
    for m, g in enumerate(heads):
        wv[:, 65 * m:65 * m + 64] = W_kqv[g][:, 128:192]
        bv[0, 65 * m:65 * m + 64] = b_kqv[g][128:192]
        bv[0, 65 * m + 64] = 1.0
    wproj = np.ascontiguousarray(
        W_proj[512 * half:512 * half + 512, :]).reshape(PAIRS, P, D)
    mask = np.triu(np.ones((P, P), np.float32))  # mask[r,c]=1 iff c>=r
    return {
        "xT": xT, "wkq": wkq.astype(BF), "bkq": bkq,
        "wv": wv.reshape(8, P, 520).astype(BF), "bv": bv,
        "wproj": wproj.astype(BF), "mask": mask.astype(BF),
    }


class SpmdRunner:
    def __init__(self, nc, n_cores=NCORES):
        install_neuronx_cc_hook()
        self.nc = nc
        self.n_cores = n_cores
        pname = nc.partition_id_tensor.name if nc.partition_id_tensor else None
        in_names, out_names, out_avals, zero_outs = [], [], [], []
        for alloc in nc.m.functions[0].allocations:
            if not isinstance(alloc, mybir.MemoryLocationSet):
                continue
            name = alloc.memorylocations[0].name
            if alloc.kind == "ExternalInput":
                if name != pname:
                    in_names.append(name)
            elif alloc.kind == "ExternalOutput":
                out_names.append(name)
                shape = tuple(alloc.tensor_shape)
                dtype = mybir.dt.np(alloc.dtype)
                out_avals.append(jax.core.ShapedArray(shape, dtype))
                zero_outs.append(np.zeros(shape, dtype))
        self.in_names, self.out_names = in_names, out_names
        self.out_avals, self.zero_outs = out_avals, zero_outs
        n_params = len(in_names)
        all_in = in_names + out_names + ([pname] if pname else [])

        def _body(*args):
            operands = list(args)
            if pname is not None:
                operands.append(partition_id_tensor())
            outs = _bass_exec_p.bind(
                *operands, out_avals=tuple(out_avals),
                in_names=tuple(all_in), out_names=tuple(out_names),
                lowering_input_output_aliases=(),
                sim_require_finite=False, sim_require_nnan=False, nc=nc)
            return tuple(outs)

        devices = jax.devices()[:n_cores]
        self.mesh = Mesh(np.asarray(devices), ("core",))
        nin = n_params + len(out_names)
        self.sharded = jax.jit(
            shard_map(_body, mesh=self.mesh,
                      in_specs=(PartitionSpec("core"),) * nin,
                      out_specs=(PartitionSpec("core"),) * len(out_names),
                      check_rep=False),
            keep_unused=True)
        self._dev_args = None

    def put(self, in_maps):
        n = self.n_cores
        arrs = [np.concatenate([np.asarray(in_maps[c][nm]) for c in range(n)],
                               axis=0) for nm in self.in_names]
        arrs += [np.zeros((n * z.shape[0], *z.shape[1:]), z.dtype)
                 for z in self.zero_outs]
        sh = jax.sharding.NamedSharding(self.mesh, PartitionSpec("core"))
        self._dev_args = [jax.device_put(a, sh) for a in arrs]

    def run(self):
        out_arrs = self.sharded(*self._dev_args)
        jax.block_until_ready(out_arrs)
        n = self.n_cores
        return [
            {nm: np.asarray(out_arrs[i]).reshape(n, *self.out_avals[i].shape)[c]
             for i, nm in enumerate(self.out_names)}
            for c in range(n)
        ]


_CACHE = {}


def kernel(x, W_kqv, b_kqv, W_proj, b_proj):
    x = np.asarray(x, np.float32)
    W_kqv = np.asarray(W_kqv, np.float32)
    b_kqv = np.asarray(b_kqv, np.float32)
    W_proj = np.asarray(W_proj, np.float32)
    b_proj = np.asarray(b_proj, np.float32)
    if "r" not in _CACHE:
        _CACHE["r"] = SpmdRunner(build_nc(1))
    r = _CACHE["r"]
    in_maps = [prep_core_inputs(c, x, W_kqv, b_kqv, W_proj)
               for c in range(NCORES)]
    r.put(in_maps)
    res = r.run()
    y = np.empty((B, SEQ, D), np.float32)
    for b in range(B):
        y[b] = res[2 * b]["y"] + res[2 * b + 1]["y"] + b_proj[None, :]
    return y
